# revision 9
# baseline (speedup 1.0000x reference)
"""Trainium2 Bass kernel for DiagonalColCausalLinear.

Computes out[b,e,t] = sum_{s<t} x[b,e,s] * v[s] * d^(t-s) + x[b,e,t] * v2[t] + bias[t]
with d = clip(decay_value[1,0], 0.9, 1.0), v = weight, v2 = diag_weight.

Sharding: data-parallel over batch B across the 8 cores; the small parameter
tensors are replicated.

The kernel is DMA-bandwidth bound in this regime (the cost model serializes
all DMA transfers on one resource at ~360 GB/s), so x and out travel as
fp16 -- half the bytes of the f32 baseline -- and the host up/down-casts
(tolerance is 2e-2; fp16 keeps end-to-end error ~1e-3).  All PE matmuls run
in fp16 (1 cycle/row vs 4 for f32).

Device algorithm (per core; x pre-permuted on the host into G pipeline
groups, each [128 s-part, NSC chunks x EW e-cols] so every DMA is a single
large contiguous transfer):
  Chunked causal scan along the sequence axis (chunk C=128):
    - cross-chunk carries, per e-tile: 15 accumulating matmuls with R
      (N=16 moving cols -> nearly free on PE), transposed once via the PE
      so carries land as rows [c', e]
    - carry broadcast into the main PSUM via block-diagonal rank-4 matmuls
      against dpow (ones when d == 1): psum[e, c*128+t] = cy[c,e]*d^t
      (start=True zeroes the bank), then
    - within-chunk triangular matmuls accumulate on top: psum += xT_c^T T_c
    - one wide PSUM->SBUF fp16 copy per 1024 cols, alternating
      ScalarE / VectorE
  Input loads and output stores are one DMA per group on the SP ring
  (16 KB contiguous per partition row); constants ride the Pool SWDGE so
  they never block the HWDGE FIFO.
"""
import numpy as np

import concourse.bass as bass
import concourse.mybir as mybir
import concourse.tile as tile
import concourse.bacc as bacc
from concourse import bass_utils

F32 = mybir.dt.float32
F16 = mybir.dt.float16

B, E, S = 8, 2048, 2048
N_CORES = 8
PT = 128            # partition tile
C = 128             # scan chunk
NCH = S // C        # 16 chunks
NSC = S // PT       # 16 s-subchunks
G = 8               # DMA/compute pipeline groups
EG = (E // PT) // G  # e-tiles per group (2)
EW = EG * PT        # e columns per group (256)
HB = 8              # chunks per PSUM half-tile

_prog_cache: dict = {}


def _build_constants(v: np.ndarray, v2: np.ndarray, d: float):
    """Host-side (tiny) constant matrices encoding the decay structure."""
    t_local = np.arange(C)
    Tm = np.zeros((PT, NCH * C), np.float32)
    for c in range(NCH):
        s_l = np.arange(PT)
        s_glob = c * C + s_l
        diff = t_local[None, :] - s_l[:, None]
        with np.errstate(over="ignore", invalid="ignore"):
            blk = np.where(diff > 0, v[s_glob][:, None] * (d ** np.maximum(diff, 0)), 0.0)
        blk[s_l, s_l] = v2[s_glob]
        Tm[:, c * C:(c + 1) * C] = blk
    # R[p, sc*16 + c'] = v[s] * d^(c'*C - s) for chunks c' > sc (carry to chunk start)
    Rm = np.zeros((PT, NSC * NCH), np.float32)
    cc = np.arange(NCH)
    for sc in range(NSC - 1):
        s_glob = sc * C + np.arange(PT)
        expo = cc[None, :] * C - s_glob[:, None]
        with np.errstate(over="ignore", invalid="ignore"):
            Rm[:, sc * NCH:(sc + 1) * NCH] = np.where(
                cc[None, :] > sc, v[s_glob][:, None] * (d ** np.maximum(expo, 0)), 0.0)
    dpow = (d ** t_local).astype(np.float32)
    # block-diagonal carry-broadcast matrix: bd[r, u] = dpow[u-128r] on block r
    bd = np.zeros((NCH, NCH * C), np.float32)
    for r in range(NCH):
        bd[r, r * C:(r + 1) * C] = dpow
    ident = np.eye(PT, dtype=np.float32)
    return (Tm.astype(np.float16), Rm.astype(np.float16),
            bd.astype(np.float16), ident.astype(np.float16))


def _build_program(d_is_one: bool = True):
    """The program is independent of d (the decay lives in the constants)."""
    key = "prog"
    if key in _prog_cache:
        return _prog_cache[key]

    nc = bacc.Bacc("TRN2", target_bir_lowering=False, debug=False, num_devices=1)
    xg_d = nc.dram_tensor("xg", [G * PT, NSC * EW], F16, kind="ExternalInput").ap()
    tmat_d = nc.dram_tensor("tmat", [PT, NCH * C], F16, kind="ExternalInput").ap()
    rmat_d = nc.dram_tensor("rmat", [PT, NSC * NCH], F16, kind="ExternalInput").ap()
    bd_d = nc.dram_tensor("bd", [NCH, NCH * C], F16, kind="ExternalInput").ap()
    ident_d = nc.dram_tensor("ident", [PT, PT], F16, kind="ExternalInput").ap()
    out_d = nc.dram_tensor("out", [G * PT, EG * S], F16, kind="ExternalOutput").ap()

    with tile.TileContext(nc) as tc:
        with (
            tc.tile_pool(name="const", bufs=1) as cpool,
            tc.tile_pool(name="xt", bufs=1) as xtpool,
            tc.tile_pool(name="outp", bufs=3) as opool,
            tc.tile_pool(name="small", bufs=4) as spool,
            tc.tile_pool(name="psm", bufs=3, space="PSUM") as psm,
            tc.tile_pool(name="pscy", bufs=1, space="PSUM") as pscy,
            tc.tile_pool(name="pscyt", bufs=1, space="PSUM") as pscyt,
        ):
            # constants (resident) first on the SP ring so their transfers
            # run ahead of the big x streams in the DMA queue (~2 us total)
            rmat = cpool.tile([PT, NSC * NCH], F16, tag="rmat")
            nc.sync.dma_start(rmat[:, :], rmat_d[:, :])
            ident = cpool.tile([PT, PT], F16, tag="ident")
            nc.sync.dma_start(ident[:, :], ident_d[:, :])
            tmat = cpool.tile([PT, NCH * C], F16, tag="tmat")
            nc.sync.dma_start(tmat[:, :], tmat_d[:, :])
            bd = cpool.tile([NCH, NCH * C], F16, tag="bd")
            nc.sync.dma_start(bd[:, :], bd_d[:, :])

            # stream in all group slices upfront (SP HWDGE ring)
            xts = []
            for g in range(G):
                xt_sb = xtpool.tile([PT, NSC * EW], F16, tag=f"xt{g}", name=f"xt{g}")
                nc.sync.dma_start(xt_sb[:, :], xg_d[g * PT:(g + 1) * PT, :])
                xts.append(xt_sb)

            # warm the PE p-state ramp while the first x group streams in:
            # dummy matmuls on the identity keep the ramp counter running so
            # the first real groups run at full clock
            ps_warm = pscyt.tile([PT, C], F32, tag="warm")
            for _ in range(40):
                nc.tensor.matmul(ps_warm[:, :], ident[:, :], ident[:, :],
                                 start=True, stop=True)

            ncopy = 0
            for g in range(G):
                xt = xts[g]
                # carries: ps_cy[e, ii*16+c'] = sum_{s<c'*C} x[e,s]*v[s]*d^..
                ps_cy = pscy.tile([PT, EG * NCH], F32, tag="cy")
                for ii in range(EG):
                    for sc in range(NSC - 1):   # R[15] is all zero
                        nc.tensor.matmul(
                            ps_cy[:, ii * NCH:(ii + 1) * NCH],
                            xt[:, sc * EW + ii * PT: sc * EW + (ii + 1) * PT],
                            rmat[:, sc * NCH:(sc + 1) * NCH],
                            start=(sc == 0), stop=(sc == NSC - 2),
                        )
                cy_sb = spool.tile([PT, EG * NCH], F16, tag="cys")
                nc.scalar.copy(cy_sb[:, :], ps_cy[:, :])
                # transpose carries per e-tile so they land as rows [c', e]
                ps_cyT = pscyt.tile([NCH, EG * PT], F16, tag="cyT")
                for ii in range(EG):
                    nc.tensor.transpose(
                        ps_cyT[:, ii * PT:(ii + 1) * PT],
                        cy_sb[:, ii * NCH:(ii + 1) * NCH], ident[:, :])
                cyT_sb = spool.tile([NCH, EG * PT], F16, tag="cyTs")
                nc.vector.tensor_copy(cyT_sb[:, :], ps_cyT[:, :])

                out_sb = opool.tile([PT, EG * S], F16, tag="o")
                for ii in range(EG):
                    for h in range(2):          # PSUM half: chunks 8h..8h+7
                        ps = psm.tile([PT, HB * C], F32, tag="m")
                        # carry broadcast: psum[e, j*128+t] = cy[8h+j, e]*d^t
                        for q in range(2):      # per 512-col PSUM bank
                            v0 = (2 * h + q) * 512
                            nc.tensor.matmul(
                                ps[:, q * 512:(q + 1) * 512],
                                cyT_sb[:, ii * PT:(ii + 1) * PT],
                                bd[:, v0:v0 + 512],
                                start=True, stop=False, skip_group_check=True,
                            )
                        # within-chunk mains accumulate on top
                        for j in range(HB):
                            cch = HB * h + j
                            nc.tensor.matmul(
                                ps[:, j * C:(j + 1) * C],
                                xt[:, cch * EW + ii * PT: cch * EW + (ii + 1) * PT],
                                tmat[:, cch * C:(cch + 1) * C],
                                start=False, stop=True, skip_group_check=True,
                            )
                        dst = out_sb[:, ii * S + h * HB * C: ii * S + (h + 1) * HB * C]
                        if ncopy % 2 == 0:
                            nc.scalar.copy(dst, ps[:, :])
                        else:
                            nc.vector.tensor_copy(dst, ps[:, :])
                        ncopy += 1

                # ship the group's output on the SP ring (queued behind loads)
                nc.sync.dma_start(out_d[g * PT:(g + 1) * PT, :], out_sb[:, :])

    nc.compile()
    _prog_cache[key] = nc
    return nc


def kernel(x, weight, diag_weight, bias, decay_value):
    x = np.asarray(x, dtype=np.float32)
    v = np.asarray(weight, dtype=np.float32).reshape(-1)
    v2 = np.asarray(diag_weight, dtype=np.float32).reshape(-1)
    bias = np.asarray(bias, dtype=np.float32).reshape(-1)
    d = float(np.clip(np.asarray(decay_value, dtype=np.float32)[1, 0], 0.9, 1.0))

    # [B, E, S] -> per-core [G*PT, NSC*EW] fp16 with
    # xg[g*128+p, sc*EW+e_l] = x[b, g*EW+e_l, sc*128+p]
    xg = np.ascontiguousarray(
        x.reshape(B, G, EW, NSC, PT).transpose(0, 1, 4, 3, 2)
    ).reshape(B, G * PT, NSC * EW).astype(np.float16)

    Tm, Rm, bd, ident = _build_constants(v, v2, d)
    nc = _build_program()

    in_maps = [{"xg": xg[b], "tmat": Tm, "rmat": Rm, "bd": bd, "ident": ident}
               for b in range(N_CORES)]
    res = bass_utils.run_bass_kernel_spmd(nc, in_maps, core_ids=list(range(N_CORES)))
    og = np.stack([res.results[b]["out"] for b in range(N_CORES)], axis=0)
    # og[b, g*128+p, ii*S+t] = out[b, g*EW+ii*128+p, t]
    out = og.reshape(B, G, PT, EG, S).transpose(0, 1, 3, 2, 4).reshape(B, E, S)
    out = out.astype(np.float32)
    if np.any(bias):
        out = out + bias[None, None, :]
    return out


# revision 21
# speedup vs baseline: 1.0761x; 1.0761x over previous
"""Trainium2 Bass kernel for DiagonalColCausalLinear.

Computes out[b,e,t] = sum_{s<t} x[b,e,s] * v[s] * d^(t-s) + x[b,e,t] * v2[t] + bias[t]
with d = clip(decay_value[1,0], 0.9, 1.0), v = weight, v2 = diag_weight.

Sharding: data-parallel over batch B across the 8 cores; the small parameter
tensors are replicated.

The kernel is DMA-bandwidth bound in this regime (all DMA transfers
serialize on one resource at ~360 GB/s), so x and out travel as fp16 --
half the bytes of the f32 baseline -- and the host up/down-casts
(tolerance is 2e-2; fp16 keeps end-to-end error ~1e-3).  All PE matmuls
run in fp16 (1 cycle/row vs 4 for f32).  The span equals DMA lead-in +
continuous transfer + semaphore/drain tail; everything else hides under it.

Device algorithm (per core; x pre-permuted on the host into G=8 pipeline
groups, each [128 s-part, 16 chunks x 256 e-cols], so every load/store is
one large DMA with 16 KB contiguous per partition row):
  Chunked causal scan along the sequence axis (chunk C=128), O(E*S*C) work:
    - group 0's load leads the DMA queue, small constants follow under its
      shadow, then the remaining loads; stores join the queue as groups
      complete (out pool bufs=5 keeps the store chain off the critical path)
    - dummy PE matmuls on a memset tile warm the p-state ramp before the
      first real group so early groups run at full clock
    - cross-chunk carries per e-tile: 15 accumulating matmuls with R
      (N=16 moving cols -> nearly free), transposed via the PE into a
      shared PSUM carry bank
    - per 512-col PSUM bank: even banks get the carry pre-injected by one
      block-diag rank-4 PE matmul against d^t (start=True zeroes the bank)
      with mains accumulating on top + plain ScalarE copy-out; odd banks
      (d==1) skip the PE apply -- the carry is fused into the VectorE
      copy-out as a broadcast tensor_tensor add (same DVE cost as a copy)
  All DMA rides the SP ring so loads/stores share one in-order queue.
"""
import numpy as np

import concourse.bass as bass
import concourse.mybir as mybir
import concourse.tile as tile
import concourse.bacc as bacc
from concourse import bass_utils

F32 = mybir.dt.float32
F16 = mybir.dt.float16

B, E, S = 8, 2048, 2048
N_CORES = 8
PT = 128            # partition tile
C = 128             # scan chunk
NCH = S // C        # 16 chunks
NSC = S // PT       # 16 s-subchunks
G = 8               # DMA/compute pipeline groups
EG = (E // PT) // G  # e-tiles per group (2)
EW = EG * PT        # e columns per group (256)
HB = 8              # chunks per PSUM half-tile

_prog_cache: dict = {}


def _build_constants(v: np.ndarray, v2: np.ndarray, d: float):
    """Host-side (tiny) constant matrices encoding the decay structure."""
    t_local = np.arange(C)
    Tm = np.zeros((PT, NCH * C), np.float32)
    for c in range(NCH):
        s_l = np.arange(PT)
        s_glob = c * C + s_l
        diff = t_local[None, :] - s_l[:, None]
        with np.errstate(over="ignore", invalid="ignore"):
            blk = np.where(diff > 0, v[s_glob][:, None] * (d ** np.maximum(diff, 0)), 0.0)
        blk[s_l, s_l] = v2[s_glob]
        Tm[:, c * C:(c + 1) * C] = blk
    # R[p, sc*16 + c'] = v[s] * d^(c'*C - s) for chunks c' > sc (carry to chunk start)
    Rm = np.zeros((PT, NSC * NCH), np.float32)
    cc = np.arange(NCH)
    for sc in range(NSC - 1):
        s_glob = sc * C + np.arange(PT)
        expo = cc[None, :] * C - s_glob[:, None]
        with np.errstate(over="ignore", invalid="ignore"):
            Rm[:, sc * NCH:(sc + 1) * NCH] = np.where(
                cc[None, :] > sc, v[s_glob][:, None] * (d ** np.maximum(expo, 0)), 0.0)
    dpow = (d ** t_local).astype(np.float32)
    # block-diagonal carry-broadcast matrix: bd[r, u] = dpow[u-128r] on block r
    bd = np.zeros((NCH, NCH * C), np.float32)
    for r in range(NCH):
        bd[r, r * C:(r + 1) * C] = dpow
    ident = np.eye(PT, dtype=np.float32)
    return (Tm.astype(np.float16), Rm.astype(np.float16),
            bd.astype(np.float16), ident)


def _build_program(d_is_one: bool = True):
    """d==1 (the clip makes this the common case) fuses the carry add into
    the PSUM->SBUF copy-outs as broadcast tensor_tensor adds; d<1 applies
    carries via block-diagonal PE matmuls against d^t."""
    key = bool(d_is_one)
    if key in _prog_cache:
        return _prog_cache[key]

    nc = bacc.Bacc("TRN2", target_bir_lowering=False, debug=False, num_devices=1)
    xg_d = nc.dram_tensor("xg", [G * PT, NSC * EW], F16, kind="ExternalInput").ap()
    tmat_d = nc.dram_tensor("tmat", [PT, NCH * C], F16, kind="ExternalInput").ap()
    rmat_d = nc.dram_tensor("rmat", [PT, NSC * NCH], F16, kind="ExternalInput").ap()
    bd_d = nc.dram_tensor("bd", [NCH, NCH * C], F16, kind="ExternalInput").ap()
    ident_d = nc.dram_tensor("ident", [PT, PT], F32, kind="ExternalInput").ap()
    out_d = nc.dram_tensor("out", [G * PT, EG * S], F16, kind="ExternalOutput").ap()

    with tile.TileContext(nc) as tc:
        with (
            tc.tile_pool(name="const", bufs=1) as cpool,
            tc.tile_pool(name="xt", bufs=1) as xtpool,
            tc.tile_pool(name="outp", bufs=5) as opool,
            tc.tile_pool(name="small", bufs=4) as spool,
            tc.tile_pool(name="psm", bufs=7, space="PSUM") as psm,
            tc.tile_pool(name="pscy", bufs=1, space="PSUM") as pscy,
        ):
            # group 0's x stream goes first (its long transfer covers the
            # HWDGE generation time of the constants that follow, so the DMA
            # queue never idles); constants next, then the remaining groups
            xts = []
            xt_sb = xtpool.tile([PT, NSC * EW], F16, tag="xt0", name="xt0")
            nc.sync.dma_start(xt_sb[:, :], xg_d[0:PT, :])
            xts.append(xt_sb)

            rmat = cpool.tile([PT, NSC * NCH], F16, tag="rmat")
            nc.sync.dma_start(rmat[:, :], rmat_d[:, :])
            ident = cpool.tile([PT, PT], F32, tag="ident")
            nc.sync.dma_start(ident[:, :], ident_d[:, :])
            tmat = cpool.tile([PT, NCH * C], F16, tag="tmat")
            nc.sync.dma_start(tmat[:, :], tmat_d[:, :])
            bd = cpool.tile([NCH, NCH * C], F16, tag="bd")
            nc.sync.dma_start(bd[:, :], bd_d[:, :])

            for g in range(1, G):
                xt_sb = xtpool.tile([PT, NSC * EW], F16, tag=f"xt{g}", name=f"xt{g}")
                nc.sync.dma_start(xt_sb[:, :], xg_d[g * PT:(g + 1) * PT, :])
                xts.append(xt_sb)

            # warm the PE p-state ramp while the first x group streams in:
            # dummy matmuls on a zeroed SBUF tile have no DMA dependency, so
            # they start at t~0 and the first real groups run at full clock
            # (shares the carry bank, same tag -> WAW dep; PE is in-order)
            warm_sb = spool.tile([PT, PT], F16, tag="warm")
            nc.vector.memset(warm_sb[:, :], 0)
            ps_warm = pscy.tile([PT, 4 * C], F32, tag="cy")
            for _ in range(60):
                nc.tensor.matmul(ps_warm[:, 0:EG * NCH], warm_sb[:, :],
                                 warm_sb[:, :EG * NCH], start=True, stop=True)

            # per-group: 8 PSUM banks (e-tile x half x 512-col bank); the
            # carry is injected into each bank by one block-diag PE matmul
            # (rank-4 vs d^t, ones when d==1), mains accumulate on top, and
            # the PSUM->SBUF fp16 copy-outs alternate ScalarE / VectorE.
            # Store cadence is protected by the deep out pool (bufs=5), not
            # by engine balance, so every path here is the proven one.
            for g in range(G):
                xt = xts[g]
                # carry bank: cy at cols [0:32], transposed cy at [0:16,256:512]
                ps_carry = pscy.tile([PT, 4 * C], F32, tag="cy")
                for ii in range(EG):
                    for sc in range(NSC - 1):   # R[15] is all zero
                        nc.tensor.matmul(
                            ps_carry[:, ii * NCH:(ii + 1) * NCH],
                            xt[:, sc * EW + ii * PT: sc * EW + (ii + 1) * PT],
                            rmat[:, sc * NCH:(sc + 1) * NCH],
                            start=(sc == 0), stop=(sc == NSC - 2),
                        )
                cy_sb = spool.tile([PT, EG * NCH], F32, tag="cys")
                nc.scalar.copy(cy_sb[:, :], ps_carry[:, 0:EG * NCH])
                for ii in range(EG):
                    nc.tensor.transpose(
                        ps_carry[0:NCH, 2 * C + ii * PT: 2 * C + (ii + 1) * PT],
                        cy_sb[:, ii * NCH:(ii + 1) * NCH], ident[:, :])
                cyT_sb = spool.tile([NCH, EG * PT], F16, tag="cyTs")
                nc.vector.tensor_copy(cyT_sb[:, :], ps_carry[0:NCH, 2 * C:4 * C])

                out_sb = opool.tile([PT, EG * S], F16, tag="o")
                for ii in range(EG):
                  for h in range(2):            # half: chunks 8h..8h+7
                    for q in range(2):          # 512-col PSUM bank
                        k = 4 * ii + 2 * h + q
                        c0 = HB * h + 4 * q     # first chunk of this bank
                        ps = psm.tile([PT, 4 * C], F32, tag="m")
                        # even banks: carry pre-injected into PSUM by a
                        # block-diag PE matmul, plain ScalarE copy-out.
                        # odd banks (d==1): carry fused into the VectorE
                        # copy-out as a broadcast add (same DVE cost as a
                        # plain copy), freeing the PE of their applies.
                        fuse = d_is_one and k % 2 == 1
                        if not fuse:
                            # carry: psum[e, j*128+t] = cy[c0+j, e] * d^t
                            nc.tensor.matmul(
                                ps[:, :],
                                cyT_sb[:, ii * PT:(ii + 1) * PT],
                                bd[:, (2 * h + q) * 512:(2 * h + q + 1) * 512],
                                start=True, stop=False, skip_group_check=True)
                        for j in range(4):
                            cch = c0 + j
                            nc.tensor.matmul(
                                ps[:, j * C:(j + 1) * C],
                                xt[:, cch * EW + ii * PT: cch * EW + (ii + 1) * PT],
                                tmat[:, cch * C:(cch + 1) * C],
                                start=fuse, stop=True, skip_group_check=True,
                            )
                        dst = out_sb[:, ii * S + c0 * C: ii * S + (c0 + 4) * C]
                        if fuse:
                            dst3 = dst.rearrange("p (c t) -> p c t", t=C)
                            ps3 = ps[:, :].rearrange("p (c t) -> p c t", t=C)
                            cyb = cy_sb[:, ii * NCH + c0: ii * NCH + c0 + 4][:, :, None] \
                                .to_broadcast([PT, 4, C])
                            nc.vector.tensor_tensor(
                                dst3, ps3, cyb, mybir.AluOpType.add)
                        elif k % 2 == 0:
                            nc.scalar.copy(dst, ps[:, :])
                        else:
                            nc.vector.tensor_copy(dst, ps[:, :])

                # ship the group's output on the SP ring (queued behind loads)
                nc.sync.dma_start(out_d[g * PT:(g + 1) * PT, :], out_sb[:, :])

    nc.compile()
    _prog_cache[key] = nc
    return nc


def kernel(x, weight, diag_weight, bias, decay_value):
    x = np.asarray(x, dtype=np.float32)
    v = np.asarray(weight, dtype=np.float32).reshape(-1)
    v2 = np.asarray(diag_weight, dtype=np.float32).reshape(-1)
    bias = np.asarray(bias, dtype=np.float32).reshape(-1)
    d = float(np.clip(np.asarray(decay_value, dtype=np.float32)[1, 0], 0.9, 1.0))

    # [B, E, S] -> per-core [G*PT, NSC*EW] fp16 with
    # xg[g*128+p, sc*EW+e_l] = x[b, g*EW+e_l, sc*128+p]
    xg = np.ascontiguousarray(
        x.reshape(B, G, EW, NSC, PT).transpose(0, 1, 4, 3, 2)
    ).reshape(B, G * PT, NSC * EW).astype(np.float16)

    Tm, Rm, bd, ident = _build_constants(v, v2, d)
    nc = _build_program(d_is_one=(d == 1.0))

    in_maps = [{"xg": xg[b], "tmat": Tm, "rmat": Rm, "bd": bd, "ident": ident}
               for b in range(N_CORES)]
    res = bass_utils.run_bass_kernel_spmd(nc, in_maps, core_ids=list(range(N_CORES)))
    og = np.stack([res.results[b]["out"] for b in range(N_CORES)], axis=0)
    # og[b, g*128+p, ii*S+t] = out[b, g*EW+ii*128+p, t]
    out = og.reshape(B, G, PT, EG, S).transpose(0, 1, 3, 2, 4).reshape(B, E, S)
    out = out.astype(np.float32)
    if np.any(bias):
        out = out + bias[None, None, :]
    return out
